# revision 18
# baseline (speedup 1.0000x reference)
"""Trainium2 Bass kernel for nn_Attention_56736517980223.

Full-input contract: kernel(**inputs) takes the unsharded inputs and returns
the full [2, 2048, 2048] attention output. Sharding: batch-data-parallel (2) x
tensor-parallel over heads (4): core c handles batch c//4 and heads
8*(c%4)..8*(c%4)+7 (2 KV heads). Each core emits a partial [2048, 2048]
x@wo contribution for its batch (bf16); the host sums 4 partials per batch.

v2 structure (vs v1): all inputs are pre-permuted on the host into the exact
SBUF tile layouts so every load is ONE large dma_start with 16KB/partition
lines (the Sync sequencer pays ~700ns per dma_start, so few+large wins); V is
produced directly in [token, dim] layout by token-tiled matmuls (weights = x
chunk, moving = wv) instead of DMA-XBAR transposes; wo output drains into
[128, 2048] bf16 staging tiles stored with one 1MB dma_start each; wo drains
are priority-demoted so the row-packed score matmul pairs stay adjacent on
the PE; diag-tile exps run as a single ScalarE instruction via a shifted
odd-head score layout.
"""

import numpy as np

DIM = 2048
N_HEADS = 32
N_KV_HEADS = 8
HEAD_DIM = 64
BATCH = 2
SEQ = 2048
N_CORES = 8
TPG = 4              # tensor-parallel group size (cores per batch)
HPC = 8              # q heads per core
CHUNK = 512          # token chunk (projection streaming / q block)
KC = 128             # key chunk (scores partition dim)
NQB = SEQ // CHUNK   # 4 q blocks
NKC = SEQ // KC      # 16 key chunks
NKT = DIM // KC      # 16 contraction tiles for projections
VW = 80              # padded per-block V width (64 dims + ones + pad)
SCALE = 1.0 / np.sqrt(HEAD_DIM)

_CACHE = {}
LAST_RESULT = None


def _build(tile_types, generic):
    """Build the SPMD Bass program.

    tile_types[qc][kc] in {'full', 'diag', 'skip', 'gen'} (shared across
    batches and heads). 'diag' uses the causal affine_select; 'gen' adds a
    DMA'd mask tile (only in generic mode).
    """
    from contextlib import ExitStack
    import concourse.bass as bass
    import concourse.tile as tile
    from concourse import bacc, mybir

    F32 = mybir.dt.float32
    BF16 = mybir.dt.bfloat16
    U16 = mybir.dt.uint16
    AF = mybir.ActivationFunctionType
    ALU = mybir.AluOpType

    nc = bacc.Bacc("TRN2", target_bir_lowering=False, debug=False,
                   num_devices=N_CORES)

    # all inputs pre-permuted host-side into exact SBUF layouts
    xt = nc.dram_tensor("xt", [KC, NQB * NKT * CHUNK], BF16,
                        kind="ExternalInput").ap()
    wq = nc.dram_tensor("wq", [KC, NKT * 4 * KC], BF16,
                        kind="ExternalInput").ap()
    wkv = nc.dram_tensor("wkv", [KC, NKT * 2 * KC], BF16,
                         kind="ExternalInput").ap()
    wo = nc.dram_tensor("wo", [KC, 4 * DIM], BF16, kind="ExternalInput").ap()
    cos_q = nc.dram_tensor("cos_q", [KC, SEQ], BF16,
                           kind="ExternalInput").ap()
    sin_q = nc.dram_tensor("sin_q", [KC, SEQ], BF16,
                           kind="ExternalInput").ap()
    if generic:
        maskt = nc.dram_tensor("maskt", [SEQ, SEQ], F32,
                               kind="ExternalInput").ap()
    out = nc.dram_tensor("out", [SEQ, DIM], BF16, kind="ExternalOutput").ap()

    with tile.TileContext(nc) as tc, ExitStack() as ctx:
        persist = ctx.enter_context(tc.tile_pool(name="persist", bufs=1))
        wq_sb = persist.tile([KC, NKT * 4 * KC], BF16)
        wkv_sb = persist.tile([KC, NKT * 2 * KC], BF16)
        wo_sb = persist.tile([KC, 4 * DIM], BF16)
        cos_sb = persist.tile([KC, SEQ], BF16)
        sin_sb = persist.tile([KC, SEQ], BF16)
        # per-(pairgrp, chunk) Q tiles; per-chunk K (dup'd) and V tiles
        qt = [[persist.tile([KC, CHUNK], BF16, name=f"qt{p}_{c}")
               for c in range(NQB)] for p in range(4)]
        ktc = [[persist.tile([KC, CHUNK], BF16, name=f"kt{v}_{c}")
                for c in range(NQB)] for v in range(2)]
        vtc = [[persist.tile([KC, (CHUNK // KC) * VW], BF16,
                             name=f"vt{v}_{c}")
                for c in range(NQB)] for v in range(2)]
        attnt = [[persist.tile([KC, CHUNK], BF16, name=f"at{p}_{c}")
                  for c in range(NQB)] for p in range(4)]

        xcp = ctx.enter_context(tc.tile_pool(name="xcp", bufs=2))
        rp = ctx.enter_context(tc.tile_pool(name="rp", bufs=3))
        att = ctx.enter_context(tc.tile_pool(name="att", bufs=1))
        wos = ctx.enter_context(tc.tile_pool(name="wos", bufs=1))

        xc_tiles = {}

        def load_xc(c, pieces=1):
            xc = xcp.tile([KC, NKT * CHUNK], BF16, tag="xc", name="xc")
            w = NKT * CHUNK // pieces
            for i in range(pieces):
                nc.sync.dma_start(
                    xc[:, i * w:(i + 1) * w],
                    xt[:, c * NKT * CHUNK + i * w:
                       c * NKT * CHUNK + (i + 1) * w])
            xc_tiles[c] = xc

        # load order = need order. chunk0 lands in 4 pieces interleaved with
        # the K weights so the K chain starts after ~1MB instead of 7MB;
        # wkv is laid out K-block|V-block and wq sub0|sub1 so each is
        # gated as late as its consumer.
        xc0 = xcp.tile([KC, NKT * CHUNK], BF16, tag="xc", name="xc")
        xc_tiles[0] = xc0
        HK = NKT * KC  # 2048: K-block / V-block boundary in wkv_sb
        nc.sync.dma_start(xc0[:, 0:4 * CHUNK], xt[:, 0:4 * CHUNK])
        nc.sync.dma_start(wkv_sb[:, 0:HK], wkv[:, 0:HK])
        for i in range(1, 4):
            nc.sync.dma_start(
                xc0[:, i * 4 * CHUNK:(i + 1) * 4 * CHUNK],
                xt[:, i * 4 * CHUNK:(i + 1) * 4 * CHUNK])
        nc.sync.dma_start(cos_sb[:], cos_q)
        nc.sync.dma_start(sin_sb[:], sin_q)
        nc.sync.dma_start(wq_sb[:, 0:NKT * 2 * KC], wq[:, 0:NKT * 2 * KC])
        nc.sync.dma_start(wq_sb[:, NKT * 2 * KC:], wq[:, NKT * 2 * KC:])
        nc.sync.dma_start(wkv_sb[:, HK:], wkv[:, HK:])

        # softmax-denominator ones columns (col 64 of each 80-wide V block)
        # are constant: write them once up front, off every critical path
        for kv in range(2):
            for c in range(NQB):
                nc.gpsimd.memset(
                    vtc[kv][c].rearrange(
                        "p (m w) -> p m w", w=VW)[:, :, 64:65].bitcast(U16),
                    16256)

        pps = ctx.enter_context(tc.tile_pool(name="pps", bufs=2,
                                             space="PSUM"))
        sps = ctx.enter_context(tc.tile_pool(name="sps", bufs=2,
                                             space="PSUM"))
        pvp = ctx.enter_context(tc.tile_pool(name="pvp", bufs=2,
                                             space="PSUM"))

        def emit_wo_m(wqc, m, final=False):
            """One [128-token, 2048-col] wo row: 4 accumulations + store."""
            stage = wos.tile([KC, DIM], BF16, tag="stage", name="stage",
                             bufs=3)
            msl = slice(m * KC, (m + 1) * KC)
            gm = slice(wqc * CHUNK + m * KC, wqc * CHUNK + (m + 1) * KC)
            for n in range(DIM // CHUNK):
                o_ps = pps.tile([KC, CHUNK], F32, tag="acc", name="o_ps")
                for p in range(4):
                    nc.tensor.matmul(
                        o_ps[:], attnt[p][wqc][:, msl],
                        wo_sb[:, p * DIM + n * CHUNK:
                              p * DIM + (n + 1) * CHUNK],
                        start=(p == 0), stop=(p == 3))
                # keep evacuations off ScalarE: wo tiles drain inside
                # attention windows where ScalarE is exp-bound
                nc.vector.tensor_copy(
                    stage[:, n * CHUNK:(n + 1) * CHUNK], o_ps[:])
                if final:
                    # tail: store per 512-col piece so the last DMA is small
                    nc.sync.dma_start(
                        out[gm, n * CHUNK:(n + 1) * CHUNK],
                        stage[:, n * CHUNK:(n + 1) * CHUNK])
            if not final:
                nc.sync.dma_start(out[gm, :], stage[:])

        wo_pending = []

        def drain_wo(k, final=False):
            # demoted priority: wo fills PE idle slots but never preempts
            # score/PV matmuls in the ready heap
            with tc.high_priority(offset=-500000):
                for _ in range(min(k, len(wo_pending))):
                    emit_wo_m(*wo_pending.pop(0), final=final)

        for qc in range(NQB):
            tsl = slice(qc * CHUNK, (qc + 1) * CHUNK)
            q0 = qc * CHUNK
            # ---------------- projection + RoPE for chunk qc ----------------
            if qc + 1 < NQB:
                load_xc(qc + 1)
            if qc == 1:
                nc.sync.dma_start(wo_sb[:], wo)
            xc = xc_tiles.pop(qc)

            # K projection: one [128 kdims, 512 tok] chain. RoPE-K and the
            # dup run on GpSimd (SBUF-only) to keep VectorE free for PSUM
            # evacuations; the PSUM evac itself goes to ScalarE, idle here.
            kb = rp.tile([KC, CHUNK], BF16, tag="kb", name="kb")
            acc = pps.tile([KC, CHUNK], F32, tag="acc", name="acc")
            for kti in range(NKT):
                nc.tensor.matmul(
                    acc[:],
                    wkv_sb[:, kti * KC:(kti + 1) * KC],
                    xc[:, kti * CHUNK:(kti + 1) * CHUNK],
                    start=(kti == 0), stop=(kti == NKT - 1))
            nc.scalar.activation(kb[:], acc[:], AF.Copy)
            for kv in range(2):
                ktx = ktc[kv][qc]
                ke = kb[64 * kv:64 * kv + 32, :]
                ko = kb[64 * kv + 32:64 * kv + 64, :]
                # cos_sb/sin_sb hold the same 32-row pattern tiled 4x, so
                # partition-matched slices exist for any 32-row operand
                ce = cos_sb[64 * kv:64 * kv + 32, tsl]
                se = sin_sb[64 * kv:64 * kv + 32, tsl]
                co = cos_sb[64 * kv + 32:64 * kv + 64, tsl]
                so = sin_sb[64 * kv + 32:64 * kv + 64, tsl]
                u1 = rp.tile([32, CHUNK], BF16, tag="u1")
                u2 = rp.tile([32, CHUNK], BF16, tag="u2")
                nc.vector.tensor_mul(u1[:], ke, ce)
                nc.vector.tensor_mul(u2[:], ko, so)
                nc.vector.tensor_sub(ktx[0:32, :], u1[:], u2[:])
                u3 = rp.tile([32, CHUNK], BF16, tag="u1")
                u4 = rp.tile([32, CHUNK], BF16, tag="u2")
                nc.vector.tensor_mul(u3[:], ke, se)
                nc.vector.tensor_mul(u4[:], ko, co)
                nc.vector.tensor_add(ktx[32:64, :], u3[:], u4[:])
                nc.vector.tensor_copy(ktx[64:128, :], ktx[0:64, :])

            for sub in range(2):  # Q substeps: heads 4*sub..4*sub+3
                qab = rp.tile([KC, 2 * CHUNK], BF16, tag="qab", name="qab")
                for half in range(2):  # evens / odds
                    acc = pps.tile([KC, CHUNK], F32, tag="acc", name="acc")
                    woff = sub * NKT * 2 * KC + half * KC
                    for kti in range(NKT):
                        nc.tensor.matmul(
                            acc[:],
                            wq_sb[:, woff + kti * 2 * KC:
                                  woff + kti * 2 * KC + KC],
                            xc[:, kti * CHUNK:(kti + 1) * CHUNK],
                            start=(kti == 0), stop=(kti == NKT - 1))
                    nc.scalar.activation(
                        qab[:, half * CHUNK:(half + 1) * CHUNK], acc[:],
                        AF.Copy)
                a = qab[:, 0:CHUNK]
                b = qab[:, CHUNK:2 * CHUNK]
                t1 = rp.tile([KC, CHUNK], BF16, tag="t1")
                t2 = rp.tile([KC, CHUNK], BF16, tag="t2")
                qa = rp.tile([KC, CHUNK], BF16, tag="qa")
                qb = rp.tile([KC, CHUNK], BF16, tag="qb")
                nc.vector.tensor_mul(t1[:], a, cos_sb[:, tsl])
                nc.vector.tensor_mul(t2[:], b, sin_sb[:, tsl])
                nc.vector.tensor_sub(qa[:], t1[:], t2[:])
                t3 = rp.tile([KC, CHUNK], BF16, tag="t1")
                t4 = rp.tile([KC, CHUNK], BF16, tag="t2")
                nc.vector.tensor_mul(t3[:], a, sin_sb[:, tsl])
                nc.vector.tensor_mul(t4[:], b, cos_sb[:, tsl])
                nc.vector.tensor_add(qb[:], t3[:], t4[:])
                for h in range(4):
                    dst = qt[2 * sub + h // 2][qc]
                    r0 = 64 * (h % 2)
                    nc.vector.tensor_copy(dst[r0:r0 + 32, :],
                                          qa[32 * h:32 * h + 32, :])
                    nc.vector.tensor_copy(dst[r0 + 32:r0 + 64, :],
                                          qb[32 * h:32 * h + 32, :])

            # V projection, token-tiled: weights = x chunk [128 D, 128 tok],
            # moving = wv [128 D, 128 vdim] -> V lands directly as
            # [token, vdim]; no transpose needed. Emitted after Q so the
            # first scores (which gate exp/PV) are not delayed.
            for m in range(CHUNK // KC):
                vac = pps.tile([KC, CHUNK], F32, tag="acc", name="vac")
                for kti in range(NKT):
                    nc.tensor.matmul(
                        vac[:, 0:KC],
                        xc[:, kti * CHUNK + m * KC:
                           kti * CHUNK + (m + 1) * KC],
                        wkv_sb[:, NKT * KC + kti * KC:
                               NKT * KC + (kti + 1) * KC],
                        start=(kti == 0), stop=(kti == NKT - 1))
                for kv in range(2):
                    vtx = vtc[kv][qc]
                    nc.vector.tensor_copy(
                        vtx[:, m * VW:m * VW + 64],
                        vac[:, kv * 64:kv * 64 + 64])

            # ---------------- attention for q block qc ----------------
            # per kci: emit the row-packed score pair FIRST, then exp /
            # affine, then the previous kci's PV matmuls. PV thus has a
            # later priority than the next score pair, so the pair stays
            # adjacent on the PE and runs concurrently (64-row tiles).
            kcs = [k for k in range(NKC) if tile_types[qc][k] != 'skip']
            for p in range(4):
                qtile = qt[p][qc]
                kv = p // 2
                pve = pvp.tile([65, CHUNK], F32, tag="pv", name="pve")
                pvo = pvp.tile([65, CHUNK], F32, tag="pv", name="pvo")
                pv_prev = None
                for i, kci in enumerate(kcs):
                    k0 = kci * KC
                    ci = kci // (CHUNK // KC)
                    off = (kci % (CHUNK // KC)) * KC
                    ktx = ktc[kv][ci]
                    ty = tile_types[qc][kci]
                    st = (i == 0)
                    sp = (i == len(kcs) - 1)
                    # diag tiles only need q >= k0: shrink to cols
                    # [w0:CHUNK) (earlier cols are fully masked)
                    w0 = max(0, k0 - q0) if ty == 'diag' else 0
                    W = CHUNK - w0
                    s = sps.tile([KC, 2 * CHUNK], F32, tag="s", name="s")
                    # odd head scores live at cols [512, 512+W) so the two
                    # heads' live columns are contiguous -> one exp instr
                    nc.tensor.matmul(
                        s[:, w0:CHUNK], ktx[0:64, off:off + KC],
                        qtile[0:64, w0:CHUNK], start=True, stop=True)
                    nc.tensor.matmul(
                        s[:, CHUNK:CHUNK + W], ktx[64:128, off:off + KC],
                        qtile[64:128, w0:CHUNK], start=True, stop=True)
                    ex = att.tile([KC, 2 * CHUNK], BF16,
                                  tag="ex", name="ex", bufs=3)
                    if ty == 'gen':
                        mt = att.tile([KC, CHUNK], F32, tag="mt",
                                      name="mt", bufs=4)
                        nc.sync.dma_start(
                            mt[:], maskt[k0:k0 + KC, q0:q0 + CHUNK])
                        for hh in range(2):
                            csl = slice(hh * CHUNK, (hh + 1) * CHUNK)
                            tm = att.tile([KC, CHUNK], F32, tag="tm",
                                          name="tm", bufs=4)
                            nc.vector.scalar_tensor_tensor(
                                tm[:], s[:, csl], SCALE, mt[:],
                                op0=ALU.mult, op1=ALU.add)
                            nc.scalar.activation(ex[:, csl], tm[:], AF.Exp)
                    else:
                        nc.scalar.activation(ex[:, w0:CHUNK + W],
                                             s[:, w0:CHUNK + W], AF.Exp,
                                             scale=SCALE)
                    if ty == 'diag':
                        nc.gpsimd.affine_select(
                            out=ex[:, w0:CHUNK], in_=ex[:, w0:CHUNK],
                            compare_op=ALU.is_ge, fill=0.0,
                            base=q0 + w0 - k0,
                            channel_multiplier=-1,
                            pattern=[[1, W]])
                        nc.gpsimd.affine_select(
                            out=ex[:, CHUNK:CHUNK + W],
                            in_=ex[:, CHUNK:CHUNK + W],
                            compare_op=ALU.is_ge, fill=0.0,
                            base=q0 + w0 - k0,
                            channel_multiplier=-1,
                            pattern=[[1, W]])
                    if pv_prev is not None:
                        pv_prev()
                    oslc = off // KC * VW

                    def pv_emit(vtx=vtc[kv][ci], oslc=oslc, w0=w0, W=W,
                                ex=ex, st=st, sp=sp):
                        nc.tensor.matmul(
                            pve[:, w0:CHUNK], vtx[:, oslc:oslc + 65],
                            ex[:, w0:CHUNK], start=st, stop=sp)
                        nc.tensor.matmul(
                            pvo[:, w0:CHUNK], vtx[:, oslc:oslc + 65],
                            ex[:, CHUNK:CHUNK + W], start=st, stop=sp)
                    pv_prev = pv_emit
                pv_prev()
                # normalization: one reciprocal + one broadcast per head
                # pair; reciprocal reads the PSUM denominator row directly
                srow = att.tile([1, 2 * CHUNK], F32, tag="srow",
                                name="srow", bufs=2)
                rec = att.tile([1, 2 * CHUNK], F32, tag="rec",
                               name="rec", bufs=2)
                bc = att.tile([64, 2 * CHUNK], F32, tag="bc",
                              name="bc", bufs=2)
                nc.vector.tensor_copy(srow[:, 0:CHUNK], pve[64:65, :])
                nc.vector.tensor_copy(srow[:, CHUNK:], pvo[64:65, :])
                nc.vector.reciprocal_approx_fast(rec[:], srow[:])
                nc.gpsimd.partition_broadcast(bc[:], rec[:])
                nc.vector.tensor_mul(
                    attnt[p][qc][0:64, :], pve[0:64, :], bc[:, 0:CHUNK])
                nc.vector.tensor_mul(
                    attnt[p][qc][64:128, :], pvo[0:64, :], bc[:, CHUNK:])
                # previous block's wo rows fill the PE while ScalarE
                # works through this block's exp backlog; pair 0's slot is
                # skipped so a ready row is left to carry the PE through
                # the next block transition
                drain_wo((0, 1, 1, 2)[p])

            # defer this block's output projection into the next block's
            # attention windows (final block drains below)
            wo_pending.extend((qc, m) for m in range(CHUNK // KC))

        drain_wo(len(wo_pending), final=True)

    nc.compile()
    return nc


def _classify(mask):
    """Classify (qc, kc) tiles. Returns (tile_types, generic)."""
    masked = mask <= -1e8
    zero = mask == 0.0
    tri = np.tril(np.ones((SEQ, SEQ), dtype=bool))  # keep where q >= k
    causal = bool(np.all(zero | masked)) and bool(
        np.array_equal(~masked, tri))
    types = [[None] * NKC for _ in range(NQB)]
    if bool(np.all(zero)):
        for qc in range(NQB):
            for kc in range(NKC):
                types[qc][kc] = 'full'
        return types, False
    if causal:
        for qc in range(NQB):
            q0, q1 = qc * CHUNK, qc * CHUNK + CHUNK - 1
            for kc in range(NKC):
                k0, k1 = kc * KC, kc * KC + KC - 1
                if q0 >= k1:
                    types[qc][kc] = 'full'
                elif q1 < k0:
                    types[qc][kc] = 'skip'
                else:
                    types[qc][kc] = 'diag'
        return types, False
    for qc in range(NQB):
        sub_q = slice(qc * CHUNK, (qc + 1) * CHUNK)
        for kc in range(NKC):
            sub = mask[sub_q, kc * KC:(kc + 1) * KC]
            if np.all(sub == 0.0):
                types[qc][kc] = 'full'
            elif np.all(sub <= -1e8):
                types[qc][kc] = 'skip'
            else:
                types[qc][kc] = 'gen'
    return types, True


def kernel(x, freqs_cos, freqs_sin, mask, wq, wk, wv, wo, cache_k, cache_v,
           start_pos):
    global LAST_RESULT
    from concourse import bass_utils

    x = np.asarray(x, dtype=np.float32)
    freqs_cos = np.asarray(freqs_cos, dtype=np.float32)
    freqs_sin = np.asarray(freqs_sin, dtype=np.float32)
    mask = np.asarray(mask, dtype=np.float32)
    wq = np.asarray(wq, dtype=np.float32)
    wk = np.asarray(wk, dtype=np.float32)
    wv = np.asarray(wv, dtype=np.float32)
    wo = np.asarray(wo, dtype=np.float32)
    assert int(start_pos) == 0, "kernel assumes start_pos == 0"

    tile_types, generic = _classify(mask)
    key = (tuple(tuple(r) for r in tile_types), generic)
    if key not in _CACHE:
        _CACHE[key] = _build(tile_types, generic)
    nc = _CACHE[key]

    import ml_dtypes
    bf16 = ml_dtypes.bfloat16
    cos_q = np.ascontiguousarray(
        np.tile(freqs_cos.T, (4, 1))).astype(bf16)  # [128, 2048]
    sin_q = np.ascontiguousarray(np.tile(freqs_sin.T, (4, 1))).astype(bf16)
    maskt = np.ascontiguousarray(mask.T) if generic else None

    ev = np.arange(0, HEAD_DIM, 2)
    od = np.arange(1, HEAD_DIM, 2)
    in_maps = []
    for c in range(N_CORES):
        b = c // TPG
        g = c % TPG
        heads = [HPC * g + i for i in range(HPC)]
        kvh = [2 * g, 2 * g + 1]
        # x chunk layout: [128, chunk, ktile, 512] so each chunk is one
        # contiguous 16KB-per-partition DMA
        xt = np.ascontiguousarray(
            x[b].T.reshape(NKT, KC, NQB, CHUNK).transpose(1, 2, 0, 3)
            .reshape(KC, NQB * NKT * CHUNK)).astype(bf16)
        # wq layout: [128, sub, ktile, 256] (sub-major: sub0 block is one
        # contiguous 1MB load)
        wq_subs = []
        for sub in range(2):
            hs = heads[4 * sub:4 * sub + 4]
            cols = np.concatenate(
                [np.concatenate([h * HEAD_DIM + ev for h in hs]),
                 np.concatenate([h * HEAD_DIM + od for h in hs])])
            wq_subs.append(
                wq[:, cols].reshape(NKT, KC, 2 * KC).transpose(1, 0, 2)
                .reshape(KC, NKT * 2 * KC))
        wq_l = np.ascontiguousarray(
            np.concatenate(wq_subs, axis=1)).astype(bf16)
        # wkv layout: K block [128, ktile, 128] then V block [128, ktile,
        # 128] so the K weights land first
        kcols = np.concatenate(
            [kvh[0] * HEAD_DIM + ev, kvh[0] * HEAD_DIM + od,
             kvh[1] * HEAD_DIM + ev, kvh[1] * HEAD_DIM + od])
        vcols = np.concatenate(
            [np.arange(kvh[0] * HEAD_DIM, (kvh[0] + 1) * HEAD_DIM),
             np.arange(kvh[1] * HEAD_DIM, (kvh[1] + 1) * HEAD_DIM)])
        wk_l = wk[:, kcols].reshape(NKT, KC, KC).transpose(1, 0, 2) \
            .reshape(KC, NKT * KC)
        wv_l = wv[:, vcols].reshape(NKT, KC, KC).transpose(1, 0, 2) \
            .reshape(KC, NKT * KC)
        wkv_l = np.ascontiguousarray(
            np.concatenate([wk_l, wv_l], axis=1)).astype(bf16)
        wo_shard = wo[heads[0] * HEAD_DIM:(heads[-1] + 1) * HEAD_DIM, :]
        wo_l = np.ascontiguousarray(
            wo_shard.reshape(4, KC, DIM).transpose(1, 0, 2)
            .reshape(KC, 4 * DIM)).astype(bf16)
        m = {"xt": xt, "cos_q": cos_q, "sin_q": sin_q,
             "wq": wq_l, "wkv": wkv_l, "wo": wo_l}
        if generic:
            m["maskt"] = maskt
        in_maps.append(m)

    res = bass_utils.run_bass_kernel_spmd(nc, in_maps, list(range(N_CORES)))
    LAST_RESULT = res
    outs = []
    for b in range(BATCH):
        total = np.zeros((SEQ, DIM), dtype=np.float32)
        for g in range(TPG):
            total += np.asarray(res.results[b * TPG + g]["out"],
                                dtype=np.float32)
        outs.append(total)
    return np.stack(outs, axis=0)
